# revision 17
# baseline (speedup 1.0000x reference)
"""Distributed multi-head attention for Trainium2 (8 NeuronCores).

Problem: nn_MultiHeadAttention (B=2, S=2048, D=1024, H=16, HD=64), f32.

Sharding: tensor parallel over heads — core c owns heads {2c, 2c+1}
(feature slice [128c, 128c+128)) and processes BOTH batches for them.
The output projection is sequence-parallel: one 8-core AllToAll per
batch exchanges 256-token blocks of the per-head attention outputs
(full 128 feature rows), after which core c holds all 1024 attention
features for tokens [256c, 256c+256) of each batch and contracts them
against Wo. The batch-0 AllToAll launches at the kernel midpoint and
its output projection is woven into batch-1's attention, so only the
batch-1 exchange + 256-token projection remain on the tail.

Matmuls run in bf16 (f32 PSUM accumulate). Key Trainium2 facts shaping
the implementation (HW-measured here):
  - K=64 matmuls stream at ~2 cyc/col vs 1 for K=128, so the scores
    matmuls use per-head zero-padded KT tiles (K=128, zeros kill the
    other head's contribution; QT needs no masking).
  - N=128 matmuls are LDWEIGHTS-bound (~238ns for 66ns of streaming),
    so V is projected feature-major exactly like Q/K (weights
    stationary, N=512 token streams) and transposed to token-major
    via the DMA XBAR (16x128 bf16 tiles, HWDGE-only, no PE cost).
  - Every sync-engine dma_start costs ~0.6us of sequencer time, so bulk
    loads are single strided DMAs ([128, 8, 512] etc.), not per-chunk.
  - ScalarE does ONLY exp (switching activation functions reloads LUTs);
    all PSUM evacuations go through VectorE with fused bias/cast.
  - exp is done on [128, 1024] tiles (2 PSUM banks) to amortize ~290ns
    of per-instruction ACT overhead.
  - attn^T = V_aug.T @ exp accumulated over k tiles, where V_aug carries
    a ones column -> psum row 64 is the softmax denominator for free.
  - No max subtraction in softmax: scores ~ N(0,1) by construction.
  - Collectives have ~10-20us latency: batch-split A2As hide #0 under
    attention; #1 is the only exposed tail.
"""

import numpy as np

B = 2
S = 2048          # both n_q and k (per batch)
TS = B * S        # combined token axis (4096)
D = 1024          # embed dim
H = 16            # heads
HD = 64           # head dim
N_CORES = 8
GH = 2            # heads per core
GF = GH * HD      # 128 per-core head features
TB = 512          # q/projection block (and per-core output slice)
CHK = S // N_CORES  # 256-token a2a chunk per batch
NKT = S // 128    # 16 k tiles per batch
NP = NKT // 2     # k-tile pairs (wide 1024-col exp tiles)
ECH = D // 128    # 8 contraction chunks of the embed dim

_CACHE = {}
MM_DTYPE = "bf16"  # "bf16" or "f32r"


def _build():
    import concourse.bacc as bacc
    import concourse.tile as tile
    from concourse import mybir

    F32 = mybir.dt.float32
    MDT = mybir.dt.bfloat16 if MM_DTYPE == "bf16" else mybir.dt.float32r
    Act = mybir.ActivationFunctionType

    nc = bacc.Bacc("TRN2", target_bir_lowering=False, debug=False,
                   num_devices=N_CORES)

    # ---- kernel I/O ----
    xqT = nc.dram_tensor("xqT", [D, TS], MDT, kind="ExternalInput")
    xkT = nc.dram_tensor("xkT", [D, TS], MDT, kind="ExternalInput")
    xvT = nc.dram_tensor("xvT", [D, TS], MDT, kind="ExternalInput")
    wqT = nc.dram_tensor("wqT", [128, ECH, GF], MDT, kind="ExternalInput")
    wkT = nc.dram_tensor("wkT", [128, ECH, GF], MDT, kind="ExternalInput")
    wvT = nc.dram_tensor("wvT", [128, ECH, GF], MDT, kind="ExternalInput")
    woT = nc.dram_tensor("woT", [128, ECH, D], MDT, kind="ExternalInput")

    bq_d = nc.dram_tensor("bq", [128, 1], F32, kind="ExternalInput")
    bk_d = nc.dram_tensor("bk", [128, 1], F32, kind="ExternalInput")
    bv_d = nc.dram_tensor("bv", [128, 1], F32, kind="ExternalInput")
    kmask_d = nc.dram_tensor("kmask", [128, GH], F32, kind="ExternalInput")
    kbm_d = nc.dram_tensor("kbm", [128, GH], F32, kind="ExternalInput")
    bo_d = nc.dram_tensor("bo", [128, D], F32, kind="ExternalInput")
    out_d = nc.dram_tensor("out", [TB, D], F32, kind="ExternalOutput")

    groups = [list(range(N_CORES))]

    with tile.TileContext(nc) as tc:
        with (
            tc.tile_pool(name="wpool", bufs=1) as wpool,
            tc.tile_pool(name="state", bufs=1) as state,
            tc.tile_pool(name="xpool", bufs=3) as xpool,
            tc.tile_pool(name="vfmp", bufs=2) as vfmp,
            tc.tile_pool(name="expp", bufs=3) as expp,
            tc.tile_pool(name="small", bufs=3) as small,
            tc.tile_pool(name="opool", bufs=2) as opool,
            tc.tile_pool(name="ps_proj", bufs=2, space="PSUM") as ps_proj,
            tc.tile_pool(name="ps_sc", bufs=2, space="PSUM") as ps_sc,
            tc.tile_pool(name="ps_at", bufs=1, space="PSUM") as ps_at,
            tc.tile_pool(name="dramp", bufs=1, space="DRAM") as dramp,
        ):
            # ---- first token block's X tiles, then weights (startup path) ----
            pre_x = {}
            for (name, xsrc), eng in zip(
                    (("k", xkT), ("q", xqT), ("v", xvT)),
                    (nc.gpsimd, nc.scalar, nc.sync)):
                xt0 = xpool.tile([128, ECH, TB], MDT, tag="x",
                                 name=f"x{name}pre")
                eng.dma_start(
                    xt0[:], xsrc[:, 0:TB].rearrange("(e p) n -> p e n", p=128))
                pre_x[name] = xt0
            wk_sb = wpool.tile([128, ECH, GF], MDT, name="wk_sb")
            nc.sync.dma_start(wk_sb[:], wkT[:])
            wq_sb = wpool.tile([128, ECH, GF], MDT, name="wq_sb")
            nc.sync.dma_start(wq_sb[:], wqT[:])
            wv_sb = wpool.tile([128, ECH, GF], MDT, name="wv_sb")
            nc.sync.dma_start(wv_sb[:], wvT[:])
            wo_sb = wpool.tile([128, ECH, D], MDT, name="wo_sb")
            bq_sb = wpool.tile([128, 1], F32, name="bq_sb")
            nc.gpsimd.dma_start(bq_sb[:], bq_d[:])
            bk_sb = wpool.tile([128, 1], F32, name="bk_sb")
            nc.gpsimd.dma_start(bk_sb[:], bk_d[:])
            bv_sb = wpool.tile([128, 1], F32, name="bv_sb")
            nc.gpsimd.dma_start(bv_sb[:], bv_d[:])
            kmask_sb = wpool.tile([128, GH], F32, name="kmask_sb")
            nc.gpsimd.dma_start(kmask_sb[:], kmask_d[:])
            kbm_sb = wpool.tile([128, GH], F32, name="kbm_sb")
            nc.gpsimd.dma_start(kbm_sb[:], kbm_d[:])
            bo_sb = wpool.tile([128, D], F32, name="bo_sb")
            ident = wpool.tile([128, 128], MDT, name="ident")
            from concourse.masks import make_identity
            make_identity(nc, ident[:])

            # ---- long-lived state ----
            QT = state.tile([128, TS], MDT, name="QT")
            AT = state.tile([128, TS], MDT, name="AT")
            # per-head zero-padded KT: rows [64h, 64h+64) hold head h's
            # K features, the other 64 rows stay zero -> scores matmuls
            # run K=128 (2x faster than K=64) with unmasked QT as rhs.
            KTp = [state.tile([128, TS], MDT, name=f"KTp{h}")
                   for h in range(GH)]

            # V: [128 tok, tok-chunk, 160]; head h's features at cols
            # [80h, 80h+64) with its ones column at 80h+64 (16-element
            # alignment required by the DMA-XBAR transpose writes).
            VT = state.tile([128, B * NKT, 2 * 80], MDT, name="VT")
            for h in range(GH):
                nc.gpsimd.memset(VT[:, :, 80 * h + HD:80 * h + HD + 1], 1.0)

            # a2a buffers + gathered attention features, per batch
            # Collective buffers: explicit internal DRAM tensors (NOT pool
            # tiles) with Shared-addr-space outputs, per the collectives
            # contract. Local pool slots intermittently corrupted the
            # early (compute-concurrent) AllToAll on cores 0-1.
            a2a_in = [nc.dram_tensor(f"cc_in{b}", [N_CORES, 128, CHK], MDT,
                                     kind="Internal")[:] for b in range(B)]
            a2a_out = [nc.dram_tensor(f"cc_out{b}", [N_CORES, 128, CHK],
                                      MDT, kind="Internal")[:]
                       for b in range(B)]
            ao = [state.tile([128, ECH, CHK], MDT, name=f"ao{b}")
                  for b in range(B)]

            def pump(filler, n=1):
                if filler is None:
                    return
                for _ in range(n):
                    try:
                        next(filler)
                    except StopIteration:
                        break

            # ---- emission helpers (PE stream order == emission order) ----
            def emit_proj_gen(b):
                """Generator: yields between small PE quanta so projection
                matmuls can be woven into ACT-paced attention streams.
                K first per block so attention can chase the projections."""
                for t in range(S // TB):
                    col = b * S + t * TB
                    csl = slice(col, col + TB)
                    for name, xsrc, w_sb in (("k", xkT, wk_sb),
                                             ("q", xqT, wq_sb),
                                             ("v", xvT, wv_sb)):
                        if b == 0 and t == 0:
                            xt = pre_x[name]
                        else:
                            xt = xpool.tile([128, ECH, TB], MDT, tag="x",
                                            name=f"x{name}{b}{t}")
                            nc.sync.dma_start(
                                xt[:],
                                xsrc[:, csl].rearrange("(e p) n -> p e n",
                                                       p=128))
                        ps = ps_proj.tile([128, TB], F32, tag="pp",
                                          name=f"ps{name}{b}{t}")
                        for e in range(ECH):
                            nc.tensor.matmul(ps[:], w_sb[:, e, :],
                                             xt[:, e, :],
                                             start=(e == 0),
                                             stop=(e == ECH - 1))
                            if e % 4 == 3:
                                yield
                        if name == "q":
                            nc.vector.tensor_scalar_add(QT[:, csl], ps[:],
                                                        bq_sb[:])
                        elif name == "k":
                            for h in range(GH):
                                nc.vector.tensor_scalar(
                                    KTp[h][:, csl], ps[:],
                                    kmask_sb[:, h:h + 1], kbm_sb[:, h:h + 1],
                                    op0=mybir.AluOpType.mult,
                                    op1=mybir.AluOpType.add)
                        else:
                            # feature-major V + bias, then PE-transpose
                            # each 128-token chunk into VT (4 transposed
                            # chunks pack into one PSUM bank, evacuated
                            # by a single strided DVE copy)
                            vf = vfmp.tile([128, TB], MDT, tag="vf",
                                           name=f"vf{b}{t}")
                            nc.vector.tensor_scalar_add(vf[:], ps[:],
                                                        bv_sb[:])
                            yield
                            pst = ps_proj.tile([128, 4, 128], MDT,
                                               tag="pt", bufs=1,
                                               name=f"pst{b}{t}")
                            for m in range(4):
                                nc.tensor.transpose(
                                    pst[:, m, :],
                                    vf[:, m * 128:(m + 1) * 128],
                                    ident[:])
                                if m == 1:
                                    yield
                            kt0 = b * NKT + t * 4
                            for h in range(GH):
                                nc.vector.tensor_copy(
                                    VT[:, kt0:kt0 + 4,
                                       80 * h:80 * h + HD],
                                    pst[:, :, HD * h:HD * (h + 1)])
                            yield

            def emit_attn_unit(h, b, qb, filler=None, pumps=1):
                off = HD * h
                qcol = b * S + qb * TB
                qsl = slice(qcol, qcol + TB)
                pa = ps_at.tile([HD + 1, TB], F32, tag="at",
                                name=f"pa{h}{b}{qb}")
                exps = []
                for kp in range(NP):
                    pssc = ps_sc.tile([128, 2 * TB], F32, tag="sc",
                                      name=f"pssc{h}{b}{qb}{kp}")
                    for i in range(2):
                        kcol = b * S + (2 * kp + i) * 128
                        nc.tensor.matmul(
                            pssc[:, i * TB:(i + 1) * TB],
                            KTp[h][:, kcol:kcol + 128],
                            QT[:, qsl], start=True, stop=True)
                    ex = expp.tile([128, 2 * TB], MDT, tag="exp",
                                   name=f"ex{h}{b}{qb}{kp}")
                    nc.scalar.activation(ex[:], pssc[:], Act.Exp,
                                         scale=0.125)
                    exps.append(ex)
                    pump(filler, pumps)
                    if kp >= 1:
                        for i in range(2):
                            kt = 2 * (kp - 1) + i
                            nc.tensor.matmul(
                                pa[:],
                                VT[:, b * NKT + kt, 80 * h:80 * h + HD + 1],
                                exps[kp - 1][:, i * TB:(i + 1) * TB],
                                start=(kt == 0), stop=False)
                for i in range(2):
                    kt = 2 * (NP - 1) + i
                    nc.tensor.matmul(
                        pa[:], VT[:, b * NKT + kt, 80 * h:80 * h + HD + 1],
                        exps[NP - 1][:, i * TB:(i + 1) * TB],
                        start=False, stop=(i == 1))
                # normalize: attnT_h *= 1/den (broadcast over d)
                dn = small.tile([1, TB], F32, tag="rc", name=f"dn{h}{b}{qb}")
                nc.vector.tensor_copy(dn[:], pa[HD:HD + 1, :])
                bc = small.tile([HD, TB], F32, tag="bc", name=f"bc{h}{b}{qb}")
                nc.gpsimd.partition_broadcast(bc[:], dn[:])
                rc = small.tile([HD, TB], F32, tag="rc2", name=f"rc{h}{b}{qb}")
                nc.vector.reciprocal(rc[:], bc[:])
                nc.vector.tensor_mul(
                    AT[off:off + HD, qsl], pa[0:HD, :], rc[:])

            def emit_a2a_launch(b):
                # core c sends, for each dest core j, its 128 feature rows
                # for tokens (b, 256j..256j+256); receives full 1024
                # features for tokens (b, 256c..256c+256).
                nc.sync.dma_start(
                    a2a_in[b][:].rearrange("j p n -> p j n"),
                    AT[:, b * S:(b + 1) * S].rearrange("p (j n) -> p j n",
                                                       j=N_CORES))
                nc.gpsimd.collective_compute(
                    "AllToAll",
                    mybir.AluOpType.bypass,
                    replica_groups=groups,
                    ins=[a2a_in[b][:]],
                    outs=[a2a_out[b][:]],
                )

            def emit_gather(b):
                # The CC completion signal can beat the landing of the
                # last remote chunks by ~1us; only issue the gather from
                # a sync-queue position that executes much later.
                nc.sync.dma_start(
                    ao[b][:], a2a_out[b][:].rearrange("j p n -> p j n"))

            def emit_outproj_gen(b):
                for m in range(CHK // 128):
                    ot = opool.tile([128, D], F32, tag="ot",
                                    name=f"ot{b}{m}")
                    for fb in range(2):
                        fsl = slice(fb * 512, (fb + 1) * 512)
                        pso = ps_proj.tile([128, 512], F32, tag="pp",
                                           name=f"pso{b}{m}_{fb}")
                        for nq in range(ECH):
                            nc.tensor.matmul(
                                pso[:], ao[b][:, nq, m * 128:(m + 1) * 128],
                                wo_sb[:, nq, fsl],
                                start=(nq == 0), stop=(nq == ECH - 1))
                            if nq % 4 == 3:
                                yield
                        nc.vector.tensor_add(ot[:, fsl], pso[:],
                                             bo_sb[:, fsl])
                    nc.sync.dma_start(
                        out_d[CHK * b + 128 * m:CHK * b + 128 * (m + 1), :],
                        ot[:])
                    yield

            # ---- schedule: batch-major. Batch 0's projections chase into
            # its attention (emitted as filler); batch 1's projections and
            # batch 0's output projection weave into the ACT-paced
            # attention streams. The batch-0 A2A launches mid-kernel.
            def chain(*gens):
                for g in gens:
                    yield from g

            for _ in emit_proj_gen(0):  # batch-0 projections fully first:
                pass                    # its attention must never chase them
            fill0 = emit_proj_gen(1)
            for qb in range(4):
                for h in range(GH):
                    emit_attn_unit(h, 0, qb, filler=fill0, pumps=2)
            pump(fill0, 200)  # drain any projection remainder
            nc.gpsimd.dma_start(wo_sb[:], woT[:])
            nc.gpsimd.dma_start(bo_sb[:], bo_d[:])
            emit_a2a_launch(0)  # overlaps batch-1 attention
            for qb in range(4):
                for h in range(GH):
                    emit_attn_unit(h, 1, qb, filler=None, pumps=1)
            # sync-FIFO: staging#1 executes only once batch-1 attention is
            # done, so gather#0 (behind it) has ~80us of margin past CC#0.
            emit_a2a_launch(1)
            emit_gather(0)
            for _ in emit_outproj_gen(0):  # overlaps CC#1's transfer
                pass
            # gather#1 sits behind op0's out DMAs -> margin past CC#1.
            emit_gather(1)
            for _ in emit_outproj_gen(1):
                pass

    nc.compile()
    return nc


def _mm_np_dtype():
    if MM_DTYPE == "bf16":
        import ml_dtypes
        return np.dtype(ml_dtypes.bfloat16)
    return np.float32


def _prep_inputs(Q_input, K_input, V_input, Wq, bq, Wk, bk, Wv, bv, Wo, bo):
    """Build the 8 per-core input maps (host-side sharding + transposes)."""
    f32 = np.float32
    mmdt = _mm_np_dtype()
    xT = {}
    for nm, x in (("xqT", Q_input), ("xkT", K_input), ("xvT", V_input)):
        x = np.asarray(x, f32)
        xT[nm] = np.ascontiguousarray(
            np.concatenate([x[b].T for b in range(B)], axis=1).astype(mmdt))
    Wq, Wk, Wv, Wo = (np.asarray(w, f32) for w in (Wq, Wk, Wv, Wo))
    bq, bk, bv, bo = (np.asarray(v, f32) for v in (bq, bk, bv, bo))

    def peF(wT):  # [D, F] -> [128, ECH, F] partition-major (fat descriptors)
        return np.ascontiguousarray(
            wT.reshape(ECH, 128, wT.shape[1]).transpose(1, 0, 2).astype(mmdt))

    woT_full = peF(Wo.T)
    bo_bc = np.ascontiguousarray(np.broadcast_to(bo, (128, D)))
    kmask = np.zeros((128, GH), f32)
    for h in range(GH):
        kmask[HD * h:HD * h + HD, h] = 1.0

    in_maps = []
    for c in range(N_CORES):
        hsl = slice(c * GF, (c + 1) * GF)
        in_maps.append({
            **xT,
            "wqT": peF(Wq[hsl, :].T),
            "wkT": peF(Wk[hsl, :].T),
            "wvT": peF(Wv[hsl, :].T),
            "woT": woT_full,
            "bq": np.ascontiguousarray(bq[hsl].reshape(128, 1)),
            "bk": np.ascontiguousarray(bk[hsl].reshape(128, 1)),
            "bv": np.ascontiguousarray(bv[hsl].reshape(128, 1)),
            "kmask": kmask,
            "kbm": np.ascontiguousarray(kmask * bk[hsl].reshape(128, 1)),
            "bo": bo_bc,
        })
    return in_maps


def kernel(**inputs):
    from concourse.bass_utils import run_bass_kernel_spmd

    if "nc" not in _CACHE:
        _CACHE["nc"] = _build()
    nc = _CACHE["nc"]

    in_maps = _prep_inputs(**inputs)
    res = run_bass_kernel_spmd(nc, in_maps, core_ids=list(range(N_CORES)))

    # core c holds tokens (b, 256c..256c+256) for each batch b at
    # out rows [256b, 256b+256)
    out = np.empty((B, S, D), np.float32)
    for c in range(N_CORES):
        for b in range(B):
            out[b, CHK * c:CHK * (c + 1), :] = \
                res.results[c]["out"][CHK * b:CHK * (b + 1)]
    return out
